# revision 20
# baseline (speedup 1.0000x reference)
"""Causal self-attention (B=4, T=2048, C=1024, H=16) on 8 Trainium2 cores.

Sharding: core c handles batch b = c // 2 and head group g = c % 2
(heads 8g..8g+7, i.e. a 512-wide slice of the QKV/proj feature dim).
Each core computes q/k/v projections for its slice, causal attention for
its 8 heads, and a partial output projection; the host sums the two
partials per batch (the "all-reduce after proj") and transposes back.
Partials are emitted in bf16 (halves the output DMA); the host sum is
fp32.

On-chip layout is fully transposed (feature dim on partitions, time on
the free axis) so that attention scores come out as S^T[tk, tq] and can
feed the P@V matmul without any on-chip transposes.  Softmax denominators
ride along as an extra ones-column appended to V (row 64 of the PV psum),
and 1/Z is broadcast across partitions for the normalize.  Matmuls run in
bf16 with fp32 PSUM accumulation (rel err ~4e-3 vs the fp32 reference);
scores skip the max-subtraction (|s| < ~4 for this input distribution).
Head pairs share the PE array via row groups (head-dim contraction is
only 64), causality is exploited at 128-wide granularity, and the
exp/PV chain is software-pipelined (LA=3) so the PE never waits on the
scalar engine.

Scheduling: all non-attention PE work (q/k/v projection pieces of the
NEXT time chunk, output-projection pieces of the PREVIOUS chunk) is put
on a filler queue and pumped from fixed slots INSIDE the attention
pipeline loop — right after each head-pair's first scores (covering the
softmax z-chain latency at head-pair boundaries) and every ~3 iterations
(absorbing the ACT-vs-PE rate gap).  This keeps the PE dense through
attention chunk 0 (which previously head-of-line blocked on exp) and
chunk 3 (which previously ran out of proj filler).  DMA is spread across
the sync/gpsimd/vector queues so input loads and output drains never
serialize on one queue.
"""

import sys

for _p in ("/root/.axon_site/_ro/trn_rl_repo", "/opt/trn_rl_repo"):
    if _p not in sys.path:
        sys.path.append(_p)

import numpy as np

import concourse.bass as bass
import concourse.mybir as mybir
import concourse.tile as tile
from concourse import bacc
from concourse.bass_utils import run_bass_kernel_spmd

B, T, C, H = 4, 2048, 1024, 16
HD = C // H  # 64 head dim
J = C // 2  # 512: per-core feature slice (8 heads)
P = 128
NCORES = 8
F32 = mybir.dt.float32
BF16 = mybir.dt.bfloat16
MMDT = BF16
AF = mybir.ActivationFunctionType

# V'' layout: per head 65 columns (64 v dims + ones); a PV matmul reads a
# 128-wide window starting at h*65 so that psum row 64 is the softmax sum.
VSTRIDE = 65
VFREE_PAD = 584

_cache = {}


def _build_nc():
    nc = bacc.Bacc("TRN2", target_bir_lowering=False, debug=False)

    xT = nc.declare_dram_parameter("xT", [C, T], MMDT, isOutput=False)
    wqT = nc.declare_dram_parameter("wqT", [C, J], MMDT, isOutput=False)
    wkT = nc.declare_dram_parameter("wkT", [C, J], MMDT, isOutput=False)
    wvT = nc.declare_dram_parameter("wvT", [C, J], MMDT, isOutput=False)
    wpT = nc.declare_dram_parameter("wpT", [J, C], MMDT, isOutput=False)
    bq2 = nc.declare_dram_parameter("bq2", [P, J // P], F32, isOutput=False)
    bk2 = nc.declare_dram_parameter("bk2", [P, J // P], F32, isOutput=False)
    bpe = nc.declare_dram_parameter("bpe", [P, C // P], F32, isOutput=False)
    maskp = nc.declare_dram_parameter("mask", [P, P], MMDT, isOutput=False)
    outT = nc.declare_dram_parameter("outT", [C, T], MMDT, isOutput=True)

    xT_v = xT[:, :].rearrange("(cc p) t -> p cc t", p=P)  # [128, 8, 2048]
    wqT_v = wqT[:, :].rearrange("(cc p) j -> p cc j", p=P)  # [128, 8, 512]
    wkT_v = wkT[:, :].rearrange("(cc p) j -> p cc j", p=P)
    wvT_v = wvT[:, :].rearrange("(cc p) j -> p cc j", p=P)
    wpT_v = wpT[:, :].rearrange("(jc p) e -> p jc e", p=P)  # [128, 4, 1024]
    outT_v = outT[:, :].rearrange("(ec p) t -> p ec t", p=P)  # [128, 8, 2048]

    NTC = T // 512  # 4 time chunks of 512
    NJC = J // P  # 4 feature chunks per core slice
    NCC = C // P  # 8 contraction chunks
    NEC = C // P  # 8 output feature chunks

    with tile.TileContext(nc) as tc:
        with (
            tc.tile_pool(name="persist", bufs=1) as persist,
            tc.tile_pool(name="xstream", bufs=2) as xstream,
            tc.tile_pool(name="proj_out", bufs=4) as proj_out,
            tc.tile_pool(name="ytiles", bufs=2) as ytiles,
            tc.tile_pool(name="ptiles", bufs=8) as ptiles,
            tc.tile_pool(name="ztiles", bufs=4) as ztiles,
            tc.tile_pool(name="psAC", bufs=2, space="PSUM") as psAC,
            tc.tile_pool(name="psS", bufs=2, space="PSUM") as psS,
            tc.tile_pool(name="psY", bufs=2, space="PSUM") as psY,
        ):
            # ---- persistent SBUF tensors -------------------------------
            qT_sb = persist.tile([P, NJC, T], MMDT)  # [128, 4, 2048]
            kT_sb = persist.tile([P, NJC, T], MMDT)
            v_sb = persist.tile([P, T // P, VFREE_PAD], MMDT)  # [128, 16, 584]
            bq_sb = persist.tile([P, NJC], F32)
            bk_sb = persist.tile([P, NJC], F32)
            bpe_sb = persist.tile([P, NEC], F32)
            mask_sb = persist.tile([P, P], MMDT)
            wq_sb = persist.tile([P, NCC, J], MMDT)
            wk_sb = persist.tile([P, NCC, J], MMDT)
            wv_sb = persist.tile([P, NCC, J], MMDT)
            wp_sb = persist.tile([P, NJC, C], MMDT)  # [128, 4, 1024]

            # spin the PE on junk data so the HAM clock gate is already
            # released when the first real matmuls arrive (~3.4us ramp)
            warm_junk = persist.tile([P, 512], MMDT)
            nc.vector.memset(warm_junk[:], 0.0)
            warm_ps = psS.tile([P, 2, 512], F32, tag="s01", name="warm_ps")
            for _w in range(12):
                nc.tensor.matmul(
                    warm_ps[:, _w % 2, :],
                    warm_junk[:, 0:P],
                    warm_junk[:, :],
                    start=True,
                    stop=True,
                )

            # startup loads, paced so the first projection pieces are fed
            # as early as possible: x chunk 0 in four 2-cc slices and the
            # q/k weights in per-jc column slices, alternating the sync
            # and gpsimd queues; everything else on the scalar queue.
            nc.scalar.dma_start(out=bq_sb, in_=bq2[:, :])
            nc.scalar.dma_start(out=bk_sb, in_=bk2[:, :])
            nc.scalar.dma_start(out=bpe_sb, in_=bpe[:, :])
            nc.scalar.dma_start(out=mask_sb, in_=maskp[:, :])
            nc.scalar.dma_start(out=wv_sb, in_=wvT_v)
            nc.scalar.dma_start(out=wp_sb, in_=wpT_v)
            # ones columns of V'' (row 64 of each head window) and the
            # tail beyond head 7's window: written once, never change.
            nc.vector.tensor_copy(
                v_sb[:, :, : 8 * VSTRIDE].rearrange("p t (h d) -> p t h d", d=VSTRIDE)[
                    :, :, :, HD : HD + 1
                ],
                nc.const_aps.tensor(1.0, [P, T // P, 8, 1], F32),
            )
            nc.vector.tensor_copy(
                v_sb[:, :, 8 * VSTRIDE :],
                nc.const_aps.tensor(0.0, [P, T // P, VFREE_PAD - 8 * VSTRIDE], F32),
            )

            # ---- PE work pieces (filler units) -------------------------
            xts = {}

            def load_x(tc_i):
                ts = slice(tc_i * 512, (tc_i + 1) * 512)
                xt = xstream.tile([P, NCC, 512], MMDT, tag="xt", name="xt")
                nc.sync.dma_start(out=xt[:, :4, :], in_=xT_v[:, :4, ts])
                nc.gpsimd.dma_start(out=xt[:, 4:, :], in_=xT_v[:, 4:, ts])
                xts[tc_i] = xt

            def load_startup():
                """x chunk 0 and the q/k weights, in consumption order."""
                xt = xstream.tile([P, NCC, 512], MMDT, tag="xt", name="xt")
                xts[0] = xt
                ts = slice(0, 512)
                nc.sync.dma_start(out=xt[:, 0:2, :], in_=xT_v[:, 0:2, ts])
                nc.gpsimd.dma_start(out=xt[:, 2:4, :], in_=xT_v[:, 2:4, ts])
                jsl = slice(0, P)
                nc.sync.dma_start(out=wq_sb[:, :, jsl], in_=wqT_v[:, :, jsl])
                nc.gpsimd.dma_start(out=wk_sb[:, :, jsl], in_=wkT_v[:, :, jsl])
                nc.sync.dma_start(out=xt[:, 4:6, :], in_=xT_v[:, 4:6, ts])
                nc.gpsimd.dma_start(out=xt[:, 6:8, :], in_=xT_v[:, 6:8, ts])
                for jc in range(1, NJC):
                    jsl = slice(jc * P, (jc + 1) * P)
                    nc.sync.dma_start(out=wq_sb[:, :, jsl], in_=wqT_v[:, :, jsl])
                    nc.gpsimd.dma_start(out=wk_sb[:, :, jsl], in_=wkT_v[:, :, jsl])

            def qk_piece(tc_i, jc, which):
                """q or k projection for one (time chunk, feature chunk)."""
                ts = slice(tc_i * 512, (tc_i + 1) * 512)
                jsl = slice(jc * P, (jc + 1) * P)
                xt = xts[tc_i]
                w_sb, dst, b_sb = (
                    (wq_sb, qT_sb, bq_sb) if which == "q" else (wk_sb, kT_sb, bk_sb)
                )
                ps = psAC.tile([P, 512], F32, tag="psAC", name="qk_ps")
                for cc in range(NCC):
                    nc.tensor.matmul(
                        ps[:],
                        w_sb[:, cc, jsl],
                        xt[:, cc, :],
                        start=(cc == 0),
                        stop=(cc == NCC - 1),
                    )
                nc.vector.tensor_scalar_add(dst[:, jc, ts], ps[:], b_sb[:, jc : jc + 1])

            def v_piece(tc_i, s4):
                """v projection for one 128-wide time slice."""
                t16 = tc_i * 4 + s4
                xt = xts[tc_i]
                v_ps = psAC.tile([P, 512], F32, tag="psAC", name="v_ps")
                for cc in range(NCC):
                    nc.tensor.matmul(
                        v_ps[:],
                        xt[:, cc, s4 * P : (s4 + 1) * P],
                        wv_sb[:, cc, :],
                        start=(cc == 0),
                        stop=(cc == NCC - 1),
                    )
                vrow = v_sb[:, t16, : 8 * VSTRIDE].rearrange("p (h d) -> p h d", d=VSTRIDE)
                nc.vector.tensor_copy(
                    vrow[:, :, :HD],
                    v_ps[:].rearrange("p (h d) -> p h d", d=HD),
                )

            _dmasel = [0]

            def proj_piece(qc, ec, yt):
                """output projection for one (q chunk, feature chunk)."""
                qsl = slice(qc * 512, (qc + 1) * 512)
                o_ps = psAC.tile([P, 512], F32, tag="psAC", name="o_ps")
                for jc in range(NJC):
                    nc.tensor.matmul(
                        o_ps[:],
                        wp_sb[:, jc, ec * P : (ec + 1) * P],
                        yt[:, jc, :],
                        start=(jc == 0),
                        stop=(jc == NJC - 1),
                    )
                o_sb = proj_out.tile([P, 512], MMDT, tag="osb", name="o_sb")
                nc.vector.tensor_scalar_add(o_sb[:], o_ps[:], bpe_sb[:, ec : ec + 1])
                eng = nc.sync if _dmasel[0] % 2 == 0 else nc.gpsimd
                _dmasel[0] += 1
                eng.dma_start(out=outT_v[:, ec, qsl], in_=o_sb[:])

            # ---- attention with filler pumping -------------------------
            def attention_chunk(qc, slots):
                """causal attention for q chunk qc; returns the yt tile.
                slots[ph]: closures (PE-dense work) pumped from fixed
                positions inside that head pair's pipeline loop; None
                entries skip a pump position."""
                n_kc = 4 * qc + 4
                yt = ytiles.tile([P, NJC, 512], MMDT, tag="yt", name="yt")

                for ph in range(NJC):  # head pair (2ph, 2ph+1)
                    fillq = slots[ph]

                    def pump(fillq=fillq):
                        if fillq:
                            f = fillq.pop(0)
                            if f is not None:
                                f()
                    y_ps = [
                        psY.tile([P, 512], F32, tag="psY", name="y_ps0"),
                        psY.tile([P, 512], F32, tag="psY", name="y_ps1"),
                    ]

                    def win(kc, qc=qc):
                        r = kc - 4 * qc
                        return (128 * r, 512 - 128 * r) if r >= 0 else (0, 512)

                    # software pipeline over kc PAIRS: scores for pair m
                    # (4 row-group matmuls back to back, so the PE only
                    # switches 64<->128 row config twice per pair), exp per
                    # kc, PV for pair m-LAP.  Pumps before the PV keep the
                    # PE dense; during the drain steps the pump moves after
                    # the PV so the last accumulation finishes ASAP.
                    LAP = 2
                    n_pairs = n_kc // 2

                    def scores_exp(kc):
                        off, W = win(kc)
                        s01 = psS.tile([P, 2, 512], F32, tag="s01", name="s01")
                        for i in range(2):
                            prt = slice(64 * i, 64 * i + 64)
                            nc.tensor.matmul(
                                s01[:, i, off : off + W],
                                kT_sb[prt, ph, kc * P : (kc + 1) * P],
                                qT_sb[prt, ph, qc * 512 + off : (qc + 1) * 512],
                                start=True,
                                stop=True,
                            )
                        p01 = ptiles.tile([P, 2, 512], MMDT, tag="p01", name="p01")
                        nc.scalar.activation(
                            out=p01[:, :, off : off + W],
                            in_=s01[:, :, off : off + W],
                            func=AF.Exp,
                            bias=0.0,
                            scale=float(1.0 / np.sqrt(HD)),
                        )
                        if kc - 4 * qc >= 0:
                            # only the 128-wide diagonal band needs the
                            # triangular mask; columns beyond it are fully
                            # causal-valid.
                            nc.vector.tensor_mul(
                                p01[:, :, off : off + P],
                                p01[:, :, off : off + P],
                                mask_sb[:, None, :].to_broadcast([P, 2, P]),
                            )
                        return p01

                    def pv(kc, p01):
                        off, W = win(kc)
                        for i in range(2):
                            h = 2 * ph + i
                            nc.tensor.matmul(
                                y_ps[i][:, off : off + W],
                                v_sb[:, kc, h * VSTRIDE : h * VSTRIDE + P],
                                p01[:, i, off : off + W],
                                start=(kc == 0),
                                stop=(kc == n_kc - 1),
                                skip_group_check=True,
                            )

                    p01s = {}
                    for m in range(n_pairs + LAP):
                        if m < n_pairs:
                            for kc in (2 * m, 2 * m + 1):
                                p01s[kc] = scores_exp(kc)
                            pump()
                        if m >= LAP:
                            for kc in (2 * (m - LAP), 2 * (m - LAP) + 1):
                                pv(kc, p01s.pop(kc))
                    for i in range(2):
                        # row 64 of y psum = softmax denominator.  NOTE: the
                        # custom-DVE reciprocal must NOT read PSUM directly —
                        # on hardware it returns wrong data (fine in CoreSim);
                        # stage through SBUF.
                        zraw = ztiles.tile([1, 512], F32, tag="zraw", name="zraw")
                        nc.vector.tensor_copy(zraw[:], y_ps[i][64:65, :])
                        zrec = ztiles.tile([1, 512], F32, tag="zrec", name="zrec")
                        nc.vector.reciprocal_approx_fast(zrec[:], zraw[:])
                        zb = ztiles.tile([64, 512], F32, tag="zb", name="zb")
                        nc.gpsimd.partition_broadcast(zb[:], zrec[:])
                        nc.vector.tensor_mul(
                            yt[64 * i : 64 * i + 64, ph, :],
                            y_ps[i][0:64, :],
                            zb[:],
                        )
                    # drain pumps AFTER the z-chain emission: their DVE
                    # reads must queue behind the critical zraw/normalize
                    # ops, while their matmuls fill the PE during the
                    # z-chain (and the next head pair's psY-reuse wait).
                    pump()
                    pump()
                # drain any unpumped filler
                for fillq in slots:
                    for f in fillq:
                        if f is not None:
                            f()
                    del fillq[:]
                return yt

            # ---- schedule ----------------------------------------------
            # qkv(0) runs up front (paced by the x0 DMA).  Each attention
            # chunk then pulls its filler queue:
            #   att(0): all of qkv(1)
            #   att(1): proj(0) + all of qkv(2)
            #   att(2): proj(1) + qkv(3) jc0/jc1 + v(3)
            #   att(3): qkv(3) jc2/jc3 (ahead of the head pairs that need
            #           them) + proj(2)
            # proj(3) runs at the end, overlapping the last z-chain.
            load_startup()
            for jc in range(3):
                qk_piece(0, jc, "q")
                qk_piece(0, jc, "k")
            v_piece(0, 0)
            v_piece(0, 1)
            qk_piece(0, 3, "q")
            qk_piece(0, 3, "k")
            v_piece(0, 2)
            v_piece(0, 3)

            def qk2(tc_i, jc):
                return [
                    lambda: qk_piece(tc_i, jc, "q"),
                    lambda: qk_piece(tc_i, jc, "k"),
                ]

            def vp(tc_i, s4):
                return lambda: v_piece(tc_i, s4)

            def pp(qc, ec, yt):
                return lambda: proj_piece(qc, ec, yt)

            # att(0): 4 pump positions per head pair -> all 12 qkv(1)
            # pieces, v tiles paced just ahead of the PV that reads them.
            load_x(1)
            yt0 = attention_chunk(
                0,
                [
                    qk2(1, 0) + [vp(1, 0), None],
                    qk2(1, 1) + [vp(1, 1), None],
                    qk2(1, 2) + [vp(1, 2), None],
                    qk2(1, 3) + [vp(1, 3), None],
                ],
            )

            # att(1): 6 pump positions per head pair -> proj(0) + qkv(2).
            load_x(2)
            yt1 = attention_chunk(
                1,
                [
                    [pp(0, 0, yt0)] + qk2(2, 0) + [None, pp(0, 1, yt0), vp(2, 0)],
                    [pp(0, 2, yt0)] + qk2(2, 1) + [None, pp(0, 3, yt0), vp(2, 1)],
                    [pp(0, 4, yt0)] + qk2(2, 2) + [None, pp(0, 5, yt0), vp(2, 2)],
                    [pp(0, 6, yt0)] + qk2(2, 3) + [None, pp(0, 7, yt0), vp(2, 3)],
                ],
            )

            # att(2): 8 pump positions per head pair -> proj(1) + the
            # early pieces of qkv(3) (jc0/jc1 and all four v tiles).
            load_x(3)
            yt2 = attention_chunk(
                2,
                [
                    [pp(1, 0, yt1)] + qk2(3, 0) + [None] * 3 + [pp(1, 1, yt1), None],
                    [vp(3, 0), vp(3, 1), pp(1, 2, yt1)] + [None] * 3 + [pp(1, 3, yt1), None],
                    [pp(1, 4, yt1)] + qk2(3, 1) + [None] * 3 + [pp(1, 5, yt1), None],
                    [vp(3, 2), vp(3, 3), pp(1, 6, yt1)] + [None] * 3 + [pp(1, 7, yt1), None],
                ],
            )

            # att(3): 10 pump positions per head pair; jc2 lands during
            # ph0 (needed by ph2), jc3 during ph1 (needed by ph3); the
            # last two proj(2) pieces ride ph3's post-PV drain slots so
            # they overlap the final z-chain.
            yt3 = attention_chunk(
                3,
                [
                    qk2(3, 2) + [pp(2, 0, yt2)] + [None] * 7,
                    qk2(3, 3) + [pp(2, 1, yt2)] + [None] * 7,
                    [pp(2, 2, yt2), pp(2, 3, yt2)] + [None] * 8,
                    [pp(2, 4, yt2), pp(2, 5, yt2)]
                    + [None] * 6
                    + [pp(2, 6, yt2), pp(2, 7, yt2)],
                ],
            )

            for ec in range(NEC):
                proj_piece(3, ec, yt3)

    nc.compile()
    return nc


def _get_nc():
    if "nc" not in _cache:
        _cache["nc"] = _build_nc()
    return _cache["nc"]


def _prep_in_maps(x, Wq, bq, Wk, bk, Wv, bv, Wp, bp):
    import ml_dtypes

    mm_np = ml_dtypes.bfloat16
    x = np.ascontiguousarray(np.asarray(x, dtype=np.float32))
    Wq = np.asarray(Wq, dtype=np.float32)
    Wk = np.asarray(Wk, dtype=np.float32)
    Wv = np.asarray(Wv, dtype=np.float32)
    Wp = np.asarray(Wp, dtype=np.float32)
    bq = np.asarray(bq, dtype=np.float32)
    bk = np.asarray(bk, dtype=np.float32)
    bv = np.asarray(bv, dtype=np.float32)
    bp = np.asarray(bp, dtype=np.float32)

    # triangular band mask for the 128-wide causal diagonal
    mask = (np.arange(P)[:, None] <= np.arange(P)[None, :]).astype(np.float32)

    in_maps = []
    for c in range(NCORES):
        b, g = c // 2, c % 2
        js = slice(g * J, (g + 1) * J)
        # bv folds into the proj bias: Wp[:, js] @ bv[js]; bp only on g==0.
        bpe = Wp[:, js] @ bv[js]
        if g == 0:
            bpe = bpe + bp
        in_maps.append(
            {
                "xT": np.ascontiguousarray(x[b].T.astype(mm_np)),
                "wqT": np.ascontiguousarray(Wq[js, :].T.astype(mm_np)),
                "wkT": np.ascontiguousarray(Wk[js, :].T.astype(mm_np)),
                "wvT": np.ascontiguousarray(Wv[js, :].T.astype(mm_np)),
                "wpT": np.ascontiguousarray(Wp[:, js].T.astype(mm_np)),
                "bq2": np.ascontiguousarray(bq[js].reshape(J // P, P).T),
                "bk2": np.ascontiguousarray(bk[js].reshape(J // P, P).T),
                "bpe": np.ascontiguousarray(bpe.reshape(C // P, P).T),
                "mask": mask.astype(mm_np),
                "outT": np.zeros((C, T), dtype=mm_np),
            }
        )
    return in_maps


def kernel(x, Wq, bq, Wk, bk, Wv, bv, Wp, bp, _trace=False, _ret_extra=None):
    nc = _get_nc()
    in_maps = _prep_in_maps(x, Wq, bq, Wk, bk, Wv, bv, Wp, bp)
    res = run_bass_kernel_spmd(nc, in_maps, list(range(NCORES)), trace=_trace)
    out = np.empty((B, T, C), dtype=np.float32)
    for b in range(B):
        out[b] = (
            res.results[2 * b]["outT"].astype(np.float32)
            + res.results[2 * b + 1]["outT"].astype(np.float32)
        ).T
    if _ret_extra is not None:
        _ret_extra["res"] = res
    return out
